# revision 32
# baseline (speedup 1.0000x reference)
"""Distributed Trainium2 Bass kernel for pre-LN multi-head attention.

Reference computation (per batch b of 2, seq n=2048, dim=1024, 16 heads x 64):
    xn = LayerNorm(x) * gamma + beta
    q, k = split(xn @ W_qk); v = xn @ W_v
    out = softmax(q k^T / 8) v  (per head)
    y = out @ W_out + b_out

Sharding: 8 cores = 2 batch groups x 4 sequence quarters. Core i owns batch
g=i//4, query tokens [qq*512, (qq+1)*512) with qq=i%4. Each core computes
LN + Q/K/V projections for its own 512 tokens, AllGathers K^T and V across
its 4-core group (full 2048-token K/V per batch), runs attention for its 512
queries over all 2048 keys (all 16 heads), and applies the output projection
with the full W_out — so the final output needs no inter-core reduction.
Host assembles the 8 per-core [1024, 512] y^T shards into [2, 2048, 1024].

Dataflow is kept "transposed" (feature dim on partitions, tokens on the free
axis) so every matmul contracts over the partition axis with 512-column
moving operands. Softmax normalization uses an ones-column block in the AV
stationary operand: each head's AV matmul emits colsum(exp S) on PSUM
partitions 0-63 and V^T exp(S^T) on partitions 64-127, so the divide is a
reciprocal + multiply on PSUM with no partition-axis reduction.

Matmul operands are stored bf16 (fp32 PSUM accumulation); LN statistics and
softmax run in fp32.
"""
import sys
import types

sys.path.insert(0, "/opt/trn_rl_repo")

# Register the NTFF profile hook that trn_boot skips when the image's antenv
# lacks axon_hooks, so run_bass_kernel_spmd(trace=True) can report exec time.
if "antenv.axon_hooks" not in sys.modules:
    try:
        from trn_agent_boot.trn_boot import _ntff_profile_via_ctypes

        _hook = _ntff_profile_via_ctypes("/opt/axon/libaxon_pjrt.so")
    except Exception:
        _hook = None
    _mod = types.ModuleType("antenv.axon_hooks")
    _mod.get_axon_ntff_profile_hook = lambda: _hook
    _mod.set_axon_ntff_profile_hook = lambda h: None
    sys.modules["antenv.axon_hooks"] = _mod

from contextlib import ExitStack

import ml_dtypes
import numpy as np
import concourse.bass as bass
import concourse.tile as tile
from concourse import bacc, mybir
from concourse.bass_utils import run_bass_kernel_spmd
from concourse.masks import make_identity

F32 = mybir.dt.float32
BF16 = mybir.dt.bfloat16
AF = mybir.ActivationFunctionType
ALU = mybir.AluOpType

B, N, DIM = 2, 2048, 1024
HEADS, DH = 16, 64
INNER = HEADS * DH  # 1024
SCALE = DH**-0.5
EPS = 1e-5

NCORES = 8
GROUP = 4          # cores per batch group (sequence quarters)
NQ = N // GROUP    # 512 local query tokens per core
DC = DIM // 128    # 8 dim chunks
KCH = N // 128     # 16 key chunks of 128 tokens
KPAIR = KCH // 2   # exp batches of 2 key chunks

MMDT = BF16        # matmul operand storage dtype
DEBUG_TAPS = False  # add debug DRAM outputs for kt/vones/qt/att

KT_BYTES = DIM * NQ          # elements in one K^T shard [1024, 512]
V_BYTES = NQ * INNER         # elements in one V shard  [512, 1024]
CC_LEN = KT_BYTES + V_BYTES  # per-rank collective payload
REPLICA_GROUPS = [[0, 1, 2, 3], [4, 5, 6, 7]]


def build_nc():
    nc = bacc.Bacc(num_devices=NCORES)

    x = nc.dram_tensor("x", [NQ, DIM], F32, kind="ExternalInput")
    ln_gamma = nc.dram_tensor("ln_gamma", [DIM], F32, kind="ExternalInput")
    ln_beta = nc.dram_tensor("ln_beta", [DIM], F32, kind="ExternalInput")
    w_qk = nc.dram_tensor("W_qk", [DIM, 2 * INNER], MMDT, kind="ExternalInput")
    w_v = nc.dram_tensor("W_v", [DIM, INNER], MMDT, kind="ExternalInput")
    w_out = nc.dram_tensor("W_out", [INNER, DIM], MMDT, kind="ExternalInput")
    b_out = nc.dram_tensor("b_out", [DIM], F32, kind="ExternalInput")
    out = nc.dram_tensor("out", [DIM, NQ], F32, kind="ExternalOutput")
    if DEBUG_TAPS:
        dbg_kt = nc.dram_tensor("dbg_kt", [128, 32 * NQ], MMDT, kind="ExternalOutput")
        dbg_vo = nc.dram_tensor(
            "dbg_vo", [128, KCH * 1536], MMDT, kind="ExternalOutput"
        )
        dbg_qt = nc.dram_tensor("dbg_qt", [128, DC * NQ], MMDT, kind="ExternalOutput")
        dbg_at = nc.dram_tensor("dbg_at", [128, DC * NQ], MMDT, kind="ExternalOutput")

    with tile.TileContext(nc) as tc, ExitStack() as ctx:
        pool = lambda name, bufs, **kw: ctx.enter_context(
            tc.tile_pool(name=name, bufs=bufs, **kw)
        )
        consts = pool("consts", 1)
        dram = pool("dram", 1, space="DRAM")
        qt_pool = pool("qt", 1)
        att_pool = pool("att", 1)
        small = pool("small", 8)
        stage = pool("stage", 3)
        pp = pool("pp", 2, space="PSUM")      # proj / outproj accumulators

        # ---- constants ---------------------------------------------------
        gamma_t = consts.tile([128, DC], F32)
        beta_t = consts.tile([128, DC], F32)
        bout_t = consts.tile([128, DC], F32)
        nc.sync.dma_start(gamma_t[:], ln_gamma.rearrange("(c p) -> p c", p=128))
        nc.sync.dma_start(beta_t[:], ln_beta.rearrange("(c p) -> p c", p=128))
        nc.sync.dma_start(bout_t[:], b_out.rearrange("(c p) -> p c", p=128))
        ident = consts.tile([128, 128], MMDT)
        make_identity(nc, ident[:])
        eps_sb = consts.tile([128, 1], F32)
        nc.vector.memset(eps_sb[:], EPS)

        # Collective payload split into 4 head-groups (4 heads each) so four
        # smaller AllGathers pipeline with projections and attention: group
        # hg carries K^T rows [hg*256,(hg+1)*256) and V cols likewise.
        HG_K = 2 * 128 * NQ      # K^T part elements per group
        HG_V = NQ * 256          # V part elements per group
        HG_LEN = HG_K + HG_V
        cc_ins = []
        cc_outs = []
        for hg in range(4):
            cc_i = dram.tile([HG_LEN], MMDT, name=f"cc_in{hg}")
            cc_o = dram.tile([GROUP * HG_LEN], MMDT, name=f"cc_out{hg}")
            cc_ins.append(cc_i)
            cc_outs.append(cc_o)

        # Q^T [1024, 512] as 8 chunks side by side: chunk m at cols m*512
        q_t = qt_pool.tile([128, DC * NQ], MMDT)
        # attention output^T [1024, 512], chunk c holds heads 2c, 2c+1
        att_t = att_pool.tile([128, DC * NQ], MMDT)

        with ExitStack() as proj_ctx:
            ppool = lambda name, bufs, **kw: proj_ctx.enter_context(
                tc.tile_pool(name=name, bufs=bufs, **kw)
            )
            ptr = ppool("ptr", 2, space="PSUM")  # transpose targets
            xw = ppool("xw", 1)
            x_sb = xw.tile([128, GROUP * DIM], F32)
            xn_nat = xw.tile([128, GROUP * DIM], MMDT)
            xnt = xw.tile([128, DC * NQ], MMDT)
            wqk_sb = xw.tile([128, DC * 2 * INNER], MMDT)
            wv_sb = xw.tile([128, DC * INNER], MMDT)

            # x first so LayerNorm starts immediately; K-half of W_qk and
            # W_v next (feed the first head-group's projections); Q-half last.
            for t in range(GROUP):
                nc.sync.dma_start(
                    x_sb[:, t * DIM : (t + 1) * DIM],
                    x[t * 128 : (t + 1) * 128, :],
                )
            for c in range(DC):
                nc.sync.dma_start(
                    wqk_sb[:, c * 2 * INNER + INNER : (c + 1) * 2 * INNER],
                    w_qk[c * 128 : (c + 1) * 128, INNER:],
                )
                nc.sync.dma_start(
                    wv_sb[:, c * INNER : (c + 1) * INNER],
                    w_v[c * 128 : (c + 1) * 128, :],
                )
            for c in range(DC):
                nc.sync.dma_start(
                    wqk_sb[:, c * 2 * INNER : c * 2 * INNER + INNER],
                    w_qk[c * 128 : (c + 1) * 128, :INNER],
                )

            # ---- LayerNorm on the 4 local token chunks ------------------
            for t in range(GROUP):
                xt = x_sb[:, t * DIM : (t + 1) * DIM]
                xg = xt.rearrange("p (n s) -> p n s", s=512)
                stats = small.tile([128, 2, 6], F32)
                for sgi in range(2):
                    nc.vector.bn_stats(stats[:, sgi, :], xg[:, sgi, :])
                mv = small.tile([128, 2], F32)
                nc.vector.bn_aggr(mv[:], stats[:])
                rstd = small.tile([128, 1], F32)
                nc.scalar.activation(rstd[:], mv[:, 1:2], AF.Sqrt, bias=eps_sb[:])
                nc.vector.reciprocal(rstd[:], rstd[:])
                nc.vector.tensor_scalar(
                    out=xn_nat[:, t * DIM : (t + 1) * DIM],
                    in0=xt,
                    scalar1=mv[:, 0:1],
                    scalar2=rstd[:],
                    op0=ALU.subtract,
                    op1=ALU.mult,
                )

            # ---- transpose xn to [dim, tokens], fusing gamma/beta -------
            # c-major so xnT chunk 0 completes first and projections start
            # before the whole transpose sweep finishes
            for c in range(DC):
                for t in range(GROUP):
                    pt = ptr.tile([128, 128], MMDT)
                    nc.tensor.transpose(
                        pt[:],
                        xn_nat[:, t * DIM + c * 128 : t * DIM + (c + 1) * 128],
                        ident[:],
                    )
                    nc.vector.tensor_scalar(
                        out=xnt[:, c * NQ + t * 128 : c * NQ + (t + 1) * 128],
                        in0=pt[:],
                        scalar1=gamma_t[:, c : c + 1],
                        scalar2=beta_t[:, c : c + 1],
                        op0=ALU.mult,
                        op1=ALU.add,
                    )

            # ---- per head-group: K^T + V projections, then its AllGather -
            for hg in range(4):
                for mg in range(2):
                    m = DC + 2 * hg + mg  # K^T row block (W_qk col block)
                    pq = pp.tile([128, 512], F32, tag="acc")
                    for c in range(DC):
                        nc.tensor.matmul(
                            pq[:],
                            wqk_sb[:, c * 2 * INNER + m * 128 : c * 2 * INNER + (m + 1) * 128],
                            xnt[:, c * NQ : (c + 1) * NQ],
                            start=(c == 0),
                            stop=(c == DC - 1),
                        )
                    kst = stage.tile([128, 512], MMDT, tag="stg")
                    nc.scalar.copy(kst[:], pq[:])
                    koff = mg * 128 * NQ
                    nc.sync.dma_start(
                        cc_ins[hg][koff : koff + 128 * NQ].rearrange(
                            "(p f) -> p f", f=NQ
                        ),
                        kst[:],
                    )
                for t in range(GROUP):
                    pv = pp.tile([128, 512], F32, tag="acc")
                    for c in range(DC):
                        nc.tensor.matmul(
                            pv[:, 0:256],
                            xnt[:, c * NQ + t * 128 : c * NQ + (t + 1) * 128],
                            wv_sb[:, c * INNER + hg * 256 : c * INNER + (hg + 1) * 256],
                            start=(c == 0),
                            stop=(c == DC - 1),
                        )
                    vst = stage.tile([128, 512], MMDT, tag="stg")
                    nc.scalar.copy(vst[:, 0:256], pv[:, 0:256])
                    voff = HG_K + t * 128 * 256
                    nc.sync.dma_start(
                        cc_ins[hg][voff : voff + 128 * 256].rearrange(
                            "(p f) -> p f", f=256
                        ),
                        vst[:, 0:256],
                    )
                nc.gpsimd.collective_compute(
                    "AllGather",
                    ALU.bypass,
                    replica_groups=REPLICA_GROUPS,
                    ins=[cc_ins[hg][:].opt()],
                    outs=[cc_outs[hg][:].opt()],
                )

            # ---- Q^T (cols 0..1023 of W_qk), overlaps the AllGathers -----
            for m in range(DC):
                pq = pp.tile([128, 512], F32, tag="acc")
                for c in range(DC):
                    nc.tensor.matmul(
                        pq[:],
                        wqk_sb[:, c * 2 * INNER + m * 128 : c * 2 * INNER + (m + 1) * 128],
                        xnt[:, c * NQ : (c + 1) * NQ],
                        start=(c == 0),
                        stop=(c == DC - 1),
                    )
                nc.scalar.copy(q_t[:, m * NQ : (m + 1) * NQ], pq[:])

        # ---- attention-phase SBUF (proj pools released) ------------------
        with ExitStack() as att_ctx:
            apool = lambda name, bufs, **kw: att_ctx.enter_context(
                tc.tile_pool(name=name, bufs=bufs, **kw)
            )
            kv = apool("kv", 1)
            wo_pool = apool("wo", 1)
            es_pool = apool("es", 4)
            vscr_pool = apool("vscr", 3)
            rp_pool = apool("rp", 2)
            y_pool = apool("y", 2)
            ps_s = apool("ps_s", 3, space="PSUM")

            # gathered K^T: 32 tiles [128, 512]; quarter qb, row-chunk mk
            # at cols (qb*8 + mk)*512
            kt_sb = kv.tile([128, GROUP * DC * NQ], MMDT)
            # gathered V interleaved with ones blocks so each head's AV
            # stationary operand is one contiguous 128-col block AND the V
            # data lands in contiguous 128-col runs (fast DMA):
            # chunk kc spans cols [kc*1600, (kc+1)*1600); within it, pair
            # c = h//2 occupies [c*192, c*192+192) as
            # [ones (64) | V_{2c} (64) | V_{2c+1} (64)], plus one trailing
            # ones block at 1536 for head 15.
            # Head h's lhsT = cols kc*1600 + c*192 + (h%2)*128, len 128:
            # even heads [ones | V] (AV out rows 0:64 = colsum, 64:128 =
            # data), odd heads [V | ones] (rows swapped).
            VSTR = 1600
            vones = kv.tile([128, KCH * VSTR], MMDT)

            # ones blocks are data-independent: set them before the gathers land
            for kc in range(KCH):
                ones_base = vones[:, kc * VSTR : kc * VSTR + 64]
                nc.gpsimd.memset(
                    bass.AP(
                        tensor=ones_base.tensor,
                        offset=ones_base.offset,
                        ap=[ones_base.ap[0], [192, DC + 1], [1, 64]],
                    ),
                    1.0,
                )
            # per head-group loads, in attention consumption order
            for hg in range(4):
                for qb in range(GROUP):
                    for mg in range(2):
                        src = cc_outs[hg][
                            qb * HG_LEN + mg * 128 * NQ : qb * HG_LEN
                            + (mg + 1) * 128 * NQ
                        ].rearrange("(p f) -> p f", f=NQ)
                        nc.sync.dma_start(
                            kt_sb[
                                :,
                                (qb * DC + 2 * hg + mg) * NQ : (qb * DC + 2 * hg + mg + 1)
                                * NQ,
                            ],
                            src,
                        )
                # V: contiguous DMA into scratch (fast, big descriptors),
                # then GpSimd scatters into the ones-interleaved layout.
                for qb in range(GROUP):
                    vscr = vscr_pool.tile([128, 1024], MMDT, tag="vscr")
                    for t4 in range(4):
                        off = qb * HG_LEN + HG_K + t4 * 128 * 256
                        nc.sync.dma_start(
                            vscr[:, t4 * 256 : (t4 + 1) * 256],
                            cc_outs[hg][off : off + 128 * 256].rearrange(
                                "(p f) -> p f", f=256
                            ),
                        )
                    for t4 in range(4):
                        kc = qb * 4 + t4
                        vdst = vones[
                            :,
                            kc * VSTR + 2 * hg * 192 + 64 : kc * VSTR
                            + 2 * hg * 192
                            + 128,
                        ]
                        nc.gpsimd.tensor_copy(
                            bass.AP(
                                tensor=vdst.tensor,
                                offset=vdst.offset,
                                ap=[vdst.ap[0], [192, 2], [1, 128]],
                            ),
                            vscr[:, t4 * 256 : (t4 + 1) * 256].rearrange(
                                "p (c d) -> p c d", d=128
                            ),
                        )

            wout_sb = wo_pool.tile([128, DC * DIM], MMDT)
            for c in range(DC):
                nc.sync.dma_start(
                    wout_sb[:, c * DIM : (c + 1) * DIM],
                    w_out[c * 128 : (c + 1) * 128, :],
                )

            # ---- attention: per head, 16 key chunks in pairs -------------
            for h in range(HEADS):
                hp = (h % 2) * 64  # partition base of this head's rows
                hc = h // 2        # dim-chunk index of this head's rows
                po = pp.tile([128, 512], F32, tag="acc")
                for pr in range(KPAIR):
                    pss = ps_s.tile([128, 1024], F32)
                    for j in range(2):
                        kc = 2 * pr + j
                        qb, t4 = kc // 4, kc % 4
                        lhs_k = kt_sb[
                            hp : hp + 64,
                            (qb * DC + hc) * NQ + t4 * 128 : (qb * DC + hc) * NQ
                            + (t4 + 1) * 128,
                        ]
                        rhs_q = q_t[hp : hp + 64, hc * NQ : (hc + 1) * NQ]
                        nc.tensor.matmul(
                            pss[:, j * 512 : (j + 1) * 512],
                            lhs_k,
                            rhs_q,
                            start=True,
                            stop=True,
                        )
                    es = es_pool.tile([128, 1024], MMDT, tag="es")
                    nc.scalar.activation(es[:], pss[:], AF.Exp, scale=SCALE)
                    for j in range(2):
                        kc = 2 * pr + j
                        base = kc * VSTR + hc * 192 + (h % 2) * 128
                        nc.tensor.matmul(
                            po[:],
                            vones[:, base : base + 128],
                            es[:, j * 512 : (j + 1) * 512],
                            start=(pr == 0 and j == 0),
                            stop=(pr == KPAIR - 1 and j == 1),
                        )
                # data rows at hp, colsum rows at 64-hp
                # even heads: colsum rows 0:64, data 64:128; odd heads the
                # reverse. reciprocal_approx_fast only works at partition
                # base 0, so stage the colsum there and let the multiply
                # read it cross-base (legal: PSUM + SBUF inputs).
                cb, dp = hp, 64 - hp
                recip = rp_pool.tile([128, 1024], F32, tag="recip")
                nc.vector.tensor_copy(recip[0:64, 512:1024], po[cb : cb + 64, :])
                nc.vector.reciprocal_approx_fast(
                    recip[0:64, 0:512], recip[0:64, 512:1024]
                )
                nc.vector.tensor_mul(
                    att_t[hp : hp + 64, hc * NQ : (hc + 1) * NQ],
                    po[dp : dp + 64, :],
                    recip[0:64, 0:512],
                )

            if DEBUG_TAPS:
                nc.sync.dma_start(dbg_kt[:], kt_sb[:])
                nc.sync.dma_start(dbg_vo[:], vones[:])
                nc.sync.dma_start(dbg_qt[:], q_t[:])
                nc.sync.dma_start(dbg_at[:], att_t[:])

            # ---- output projection y^T = W_out^T @ att^T + b_out ---------
            for m in range(DC):
                py = pp.tile([128, 512], F32, tag="acc")
                for c in range(DC):
                    nc.tensor.matmul(
                        py[:],
                        wout_sb[:, c * DIM + m * 128 : c * DIM + (m + 1) * 128],
                        att_t[:, c * NQ : (c + 1) * NQ],
                        start=(c == 0),
                        stop=(c == DC - 1),
                    )
                y_sb = y_pool.tile([128, 512], F32, tag="y")
                nc.vector.tensor_scalar(
                    out=y_sb[:],
                    in0=py[:],
                    scalar1=bout_t[:, m : m + 1],
                    scalar2=None,
                    op0=ALU.add,
                )
                nc.sync.dma_start(out[m * 128 : (m + 1) * 128, :], y_sb[:])

    nc.compile()
    return nc


_NC_CACHE = None


def _get_nc():
    global _NC_CACHE
    if _NC_CACHE is None:
        _NC_CACHE = build_nc()
    return _NC_CACHE


def _make_in_maps(x, ln_gamma, ln_beta, W_qk, W_v, W_out, b_out):
    mmnp = mybir.dt.np(MMDT)
    wqk = np.ascontiguousarray(np.asarray(W_qk, dtype=np.float32)).astype(mmnp)
    wv = np.ascontiguousarray(np.asarray(W_v, dtype=np.float32)).astype(mmnp)
    wo = np.ascontiguousarray(np.asarray(W_out, dtype=np.float32)).astype(mmnp)
    gamma = np.ascontiguousarray(np.asarray(ln_gamma, dtype=np.float32))
    beta = np.ascontiguousarray(np.asarray(ln_beta, dtype=np.float32))
    bout = np.ascontiguousarray(np.asarray(b_out, dtype=np.float32))
    xf = np.asarray(x, dtype=np.float32)
    in_maps = []
    for i in range(NCORES):
        g, qq = i // GROUP, i % GROUP
        in_maps.append(
            {
                "x": np.ascontiguousarray(xf[g, qq * NQ : (qq + 1) * NQ, :]),
                "ln_gamma": gamma,
                "ln_beta": beta,
                "W_qk": wqk,
                "W_v": wv,
                "W_out": wo,
                "b_out": bout,
            }
        )
    return in_maps


def run(inputs: dict, trace: bool = False):
    """Run the distributed kernel; returns (full_output, BassKernelResults)."""
    nc = _get_nc()
    in_maps = _make_in_maps(**inputs)
    res = run_bass_kernel_spmd(
        nc, in_maps, core_ids=list(range(NCORES)), trace=trace
    )
    out_full = np.empty((B, N, DIM), dtype=np.float32)
    for i in range(NCORES):
        g, qq = i // GROUP, i % GROUP
        out_full[g, qq * NQ : (qq + 1) * NQ, :] = res.results[i]["out"].T
    return out_full, res


def kernel(**inputs) -> np.ndarray:
    out, _ = run(inputs, trace=False)
    return out


# revision 33
# speedup vs baseline: 1.0823x; 1.0823x over previous
"""Distributed Trainium2 Bass kernel for pre-LN multi-head attention.

Reference computation (per batch b of 2, seq n=2048, dim=1024, 16 heads x 64):
    xn = LayerNorm(x) * gamma + beta
    q, k = split(xn @ W_qk); v = xn @ W_v
    out = softmax(q k^T / 8) v  (per head)
    y = out @ W_out + b_out

Sharding: 8 cores = 2 batch groups x 4 sequence quarters. Core i owns batch
g=i//4, query tokens [qq*512, (qq+1)*512) with qq=i%4. Each core computes
LN + Q/K/V projections for its own 512 tokens, AllGathers K^T and V across
its 4-core group (full 2048-token K/V per batch), runs attention for its 512
queries over all 2048 keys (all 16 heads), and applies the output projection
with the full W_out — so the final output needs no inter-core reduction.
Host assembles the 8 per-core [1024, 512] y^T shards into [2, 2048, 1024].

Dataflow is kept "transposed" (feature dim on partitions, tokens on the free
axis) so every matmul contracts over the partition axis with 512-column
moving operands. Softmax normalization uses an ones-column block in the AV
stationary operand: each head's AV matmul emits colsum(exp S) on PSUM
partitions 0-63 and V^T exp(S^T) on partitions 64-127, so the divide is a
reciprocal + multiply on PSUM with no partition-axis reduction.

Matmul operands are stored bf16 (fp32 PSUM accumulation); LN statistics and
softmax run in fp32.
"""
import sys
import types

sys.path.insert(0, "/opt/trn_rl_repo")

# Register the NTFF profile hook that trn_boot skips when the image's antenv
# lacks axon_hooks, so run_bass_kernel_spmd(trace=True) can report exec time.
if "antenv.axon_hooks" not in sys.modules:
    try:
        from trn_agent_boot.trn_boot import _ntff_profile_via_ctypes

        _hook = _ntff_profile_via_ctypes("/opt/axon/libaxon_pjrt.so")
    except Exception:
        _hook = None
    _mod = types.ModuleType("antenv.axon_hooks")
    _mod.get_axon_ntff_profile_hook = lambda: _hook
    _mod.set_axon_ntff_profile_hook = lambda h: None
    sys.modules["antenv.axon_hooks"] = _mod

from contextlib import ExitStack

import ml_dtypes
import numpy as np
import concourse.bass as bass
import concourse.tile as tile
from concourse import bacc, mybir
from concourse.bass_utils import run_bass_kernel_spmd
from concourse.masks import make_identity

F32 = mybir.dt.float32
BF16 = mybir.dt.bfloat16
AF = mybir.ActivationFunctionType
ALU = mybir.AluOpType

B, N, DIM = 2, 2048, 1024
HEADS, DH = 16, 64
INNER = HEADS * DH  # 1024
SCALE = DH**-0.5
EPS = 1e-5

NCORES = 8
GROUP = 4          # cores per batch group (sequence quarters)
NQ = N // GROUP    # 512 local query tokens per core
DC = DIM // 128    # 8 dim chunks
KCH = N // 128     # 16 key chunks of 128 tokens
KPAIR = KCH // 2   # exp batches of 2 key chunks

MMDT = BF16        # matmul operand storage dtype
DEBUG_TAPS = False  # add debug DRAM outputs for kt/vones/qt/att

KT_BYTES = DIM * NQ          # elements in one K^T shard [1024, 512]
V_BYTES = NQ * INNER         # elements in one V shard  [512, 1024]
CC_LEN = KT_BYTES + V_BYTES  # per-rank collective payload
REPLICA_GROUPS = [[0, 1, 2, 3], [4, 5, 6, 7]]


def build_nc():
    nc = bacc.Bacc(num_devices=NCORES)

    x = nc.dram_tensor("x", [NQ, DIM], F32, kind="ExternalInput")
    ln_gamma = nc.dram_tensor("ln_gamma", [DIM], F32, kind="ExternalInput")
    ln_beta = nc.dram_tensor("ln_beta", [DIM], F32, kind="ExternalInput")
    w_qk = nc.dram_tensor("W_qk", [DIM, 2 * INNER], MMDT, kind="ExternalInput")
    w_v = nc.dram_tensor("W_v", [DIM, INNER], MMDT, kind="ExternalInput")
    w_out = nc.dram_tensor("W_out", [INNER, DIM], MMDT, kind="ExternalInput")
    b_out = nc.dram_tensor("b_out", [DIM], F32, kind="ExternalInput")
    out = nc.dram_tensor("out", [DIM, NQ], F32, kind="ExternalOutput")
    if DEBUG_TAPS:
        dbg_kt = nc.dram_tensor("dbg_kt", [128, 32 * NQ], MMDT, kind="ExternalOutput")
        dbg_vo = nc.dram_tensor(
            "dbg_vo", [128, KCH * 1536], MMDT, kind="ExternalOutput"
        )
        dbg_qt = nc.dram_tensor("dbg_qt", [128, DC * NQ], MMDT, kind="ExternalOutput")
        dbg_at = nc.dram_tensor("dbg_at", [128, DC * NQ], MMDT, kind="ExternalOutput")

    with tile.TileContext(nc) as tc, ExitStack() as ctx:
        pool = lambda name, bufs, **kw: ctx.enter_context(
            tc.tile_pool(name=name, bufs=bufs, **kw)
        )
        consts = pool("consts", 1)
        dram = pool("dram", 1, space="DRAM")
        qt_pool = pool("qt", 1)
        att_pool = pool("att", 1)
        small = pool("small", 8)
        stage = pool("stage", 3)
        pp = pool("pp", 2, space="PSUM")      # proj / outproj accumulators

        # ---- constants ---------------------------------------------------
        gamma_t = consts.tile([128, DC], F32)
        beta_t = consts.tile([128, DC], F32)
        bout_t = consts.tile([128, DC], F32)
        nc.sync.dma_start(gamma_t[:], ln_gamma.rearrange("(c p) -> p c", p=128))
        nc.sync.dma_start(beta_t[:], ln_beta.rearrange("(c p) -> p c", p=128))
        nc.sync.dma_start(bout_t[:], b_out.rearrange("(c p) -> p c", p=128))
        ident = consts.tile([128, 128], MMDT)
        make_identity(nc, ident[:])
        eps_sb = consts.tile([128, 1], F32)
        nc.vector.memset(eps_sb[:], EPS)

        # Collective payload split into 4 head-groups (4 heads each) so four
        # smaller AllGathers pipeline with projections and attention: group
        # hg carries K^T rows [hg*256,(hg+1)*256) and V cols likewise.
        HG_K = 2 * 128 * NQ      # K^T part elements per group
        HG_V = NQ * 256          # V part elements per group
        HG_LEN = HG_K + HG_V
        cc_ins = []
        cc_outs = []
        for hg in range(4):
            cc_i = dram.tile([HG_LEN], MMDT, name=f"cc_in{hg}")
            cc_o = dram.tile([GROUP * HG_LEN], MMDT, name=f"cc_out{hg}")
            cc_ins.append(cc_i)
            cc_outs.append(cc_o)

        # Q^T [1024, 512] as 8 chunks side by side: chunk m at cols m*512
        q_t = qt_pool.tile([128, DC * NQ], MMDT)
        # attention output^T [1024, 512], chunk c holds heads 2c, 2c+1
        att_t = att_pool.tile([128, DC * NQ], MMDT)

        with ExitStack() as proj_ctx:
            ppool = lambda name, bufs, **kw: proj_ctx.enter_context(
                tc.tile_pool(name=name, bufs=bufs, **kw)
            )
            ptr = ppool("ptr", 2, space="PSUM")  # transpose targets
            xw = ppool("xw", 1)
            x_sb = xw.tile([128, GROUP * DIM], F32)
            xn_nat = xw.tile([128, GROUP * DIM], MMDT)
            xnt = xw.tile([128, DC * NQ], MMDT)
            wqk_sb = xw.tile([128, DC * 2 * INNER], MMDT)
            wv_sb = xw.tile([128, DC * INNER], MMDT)

            # x first so LayerNorm starts immediately; K-half of W_qk and
            # W_v next (feed the first head-group's projections); Q-half last.
            for t in range(GROUP):
                nc.sync.dma_start(
                    x_sb[:, t * DIM : (t + 1) * DIM],
                    x[t * 128 : (t + 1) * 128, :],
                )
            for c in range(DC):
                nc.sync.dma_start(
                    wqk_sb[:, c * 2 * INNER + INNER : (c + 1) * 2 * INNER],
                    w_qk[c * 128 : (c + 1) * 128, INNER:],
                )
                nc.sync.dma_start(
                    wv_sb[:, c * INNER : (c + 1) * INNER],
                    w_v[c * 128 : (c + 1) * 128, :],
                )
            for c in range(DC):
                nc.sync.dma_start(
                    wqk_sb[:, c * 2 * INNER : c * 2 * INNER + INNER],
                    w_qk[c * 128 : (c + 1) * 128, :INNER],
                )

            # ---- LayerNorm on the 4 local token chunks ------------------
            for t in range(GROUP):
                xt = x_sb[:, t * DIM : (t + 1) * DIM]
                xg = xt.rearrange("p (n s) -> p n s", s=512)
                stats = small.tile([128, 2, 6], F32)
                for sgi in range(2):
                    nc.vector.bn_stats(stats[:, sgi, :], xg[:, sgi, :])
                mv = small.tile([128, 2], F32)
                nc.vector.bn_aggr(mv[:], stats[:])
                rstd = small.tile([128, 1], F32)
                nc.scalar.activation(rstd[:], mv[:, 1:2], AF.Sqrt, bias=eps_sb[:])
                nc.vector.reciprocal(rstd[:], rstd[:])
                nc.vector.tensor_scalar(
                    out=xn_nat[:, t * DIM : (t + 1) * DIM],
                    in0=xt,
                    scalar1=mv[:, 0:1],
                    scalar2=rstd[:],
                    op0=ALU.subtract,
                    op1=ALU.mult,
                )

            # ---- transpose xn to [dim, tokens], fusing gamma/beta -------
            # c-major so xnT chunk 0 completes first and projections start
            # before the whole transpose sweep finishes
            for c in range(DC):
                for t in range(GROUP):
                    pt = ptr.tile([128, 128], MMDT)
                    nc.tensor.transpose(
                        pt[:],
                        xn_nat[:, t * DIM + c * 128 : t * DIM + (c + 1) * 128],
                        ident[:],
                    )
                    nc.vector.tensor_scalar(
                        out=xnt[:, c * NQ + t * 128 : c * NQ + (t + 1) * 128],
                        in0=pt[:],
                        scalar1=gamma_t[:, c : c + 1],
                        scalar2=beta_t[:, c : c + 1],
                        op0=ALU.mult,
                        op1=ALU.add,
                    )

            # ---- per head-group: K^T + V projections, then its AllGather -
            for hg in range(4):
                for mg in range(2):
                    m = DC + 2 * hg + mg  # K^T row block (W_qk col block)
                    pq = pp.tile([128, 512], F32, tag="acc")
                    for c in range(DC):
                        nc.tensor.matmul(
                            pq[:],
                            wqk_sb[:, c * 2 * INNER + m * 128 : c * 2 * INNER + (m + 1) * 128],
                            xnt[:, c * NQ : (c + 1) * NQ],
                            start=(c == 0),
                            stop=(c == DC - 1),
                        )
                    kst = stage.tile([128, 512], MMDT, tag="stg")
                    nc.scalar.copy(kst[:], pq[:])
                    koff = mg * 128 * NQ
                    nc.sync.dma_start(
                        cc_ins[hg][koff : koff + 128 * NQ].rearrange(
                            "(p f) -> p f", f=NQ
                        ),
                        kst[:],
                    )
                for t in range(GROUP):
                    pv = pp.tile([128, 512], F32, tag="acc")
                    for c in range(DC):
                        nc.tensor.matmul(
                            pv[:, 0:256],
                            xnt[:, c * NQ + t * 128 : c * NQ + (t + 1) * 128],
                            wv_sb[:, c * INNER + hg * 256 : c * INNER + (hg + 1) * 256],
                            start=(c == 0),
                            stop=(c == DC - 1),
                        )
                    vst = stage.tile([128, 512], MMDT, tag="stg")
                    nc.scalar.copy(vst[:, 0:256], pv[:, 0:256])
                    voff = HG_K + t * 128 * 256
                    nc.sync.dma_start(
                        cc_ins[hg][voff : voff + 128 * 256].rearrange(
                            "(p f) -> p f", f=256
                        ),
                        vst[:, 0:256],
                    )
                nc.gpsimd.collective_compute(
                    "AllGather",
                    ALU.bypass,
                    replica_groups=REPLICA_GROUPS,
                    ins=[cc_ins[hg][:].opt()],
                    outs=[cc_outs[hg][:].opt()],
                )

            # ---- Q^T (cols 0..1023 of W_qk), overlaps the AllGathers -----
            for m in range(DC):
                pq = pp.tile([128, 512], F32, tag="acc")
                for c in range(DC):
                    nc.tensor.matmul(
                        pq[:],
                        wqk_sb[:, c * 2 * INNER + m * 128 : c * 2 * INNER + (m + 1) * 128],
                        xnt[:, c * NQ : (c + 1) * NQ],
                        start=(c == 0),
                        stop=(c == DC - 1),
                    )
                nc.scalar.copy(q_t[:, m * NQ : (m + 1) * NQ], pq[:])

        # ---- attention-phase SBUF (proj pools released) ------------------
        with ExitStack() as att_ctx:
            apool = lambda name, bufs, **kw: att_ctx.enter_context(
                tc.tile_pool(name=name, bufs=bufs, **kw)
            )
            kv = apool("kv", 1)
            wo_pool = apool("wo", 1)
            es_pool = apool("es", 4)
            vscr_pool = apool("vscr", 3)
            rp_pool = apool("rp", 2)
            y_pool = apool("y", 2)
            ps_s = apool("ps_s", 3, space="PSUM")

            # gathered K^T: 32 tiles [128, 512]; quarter qb, row-chunk mk
            # at cols (qb*8 + mk)*512
            kt_sb = kv.tile([128, GROUP * DC * NQ], MMDT)
            # gathered V interleaved with ones blocks so each head's AV
            # stationary operand is one contiguous 128-col block AND the V
            # data lands in contiguous 128-col runs (fast DMA):
            # chunk kc spans cols [kc*1600, (kc+1)*1600); within it, pair
            # c = h//2 occupies [c*192, c*192+192) as
            # [ones (64) | V_{2c} (64) | V_{2c+1} (64)], plus one trailing
            # ones block at 1536 for head 15.
            # Head h's lhsT = cols kc*1600 + c*192 + (h%2)*128, len 128:
            # even heads [ones | V] (AV out rows 0:64 = colsum, 64:128 =
            # data), odd heads [V | ones] (rows swapped).
            VSTR = 1600
            vones = kv.tile([128, KCH * VSTR], MMDT)

            # ones blocks are data-independent: set them before the gathers land
            for kc in range(KCH):
                ones_base = vones[:, kc * VSTR : kc * VSTR + 64]
                nc.gpsimd.memset(
                    bass.AP(
                        tensor=ones_base.tensor,
                        offset=ones_base.offset,
                        ap=[ones_base.ap[0], [192, DC + 1], [1, 64]],
                    ),
                    1.0,
                )
            # per head-group loads, in attention consumption order
            for hg in range(4):
                for qb in range(GROUP):
                    for mg in range(2):
                        src = cc_outs[hg][
                            qb * HG_LEN + mg * 128 * NQ : qb * HG_LEN
                            + (mg + 1) * 128 * NQ
                        ].rearrange("(p f) -> p f", f=NQ)
                        nc.sync.dma_start(
                            kt_sb[
                                :,
                                (qb * DC + 2 * hg + mg) * NQ : (qb * DC + 2 * hg + mg + 1)
                                * NQ,
                            ],
                            src,
                        )
                # V: contiguous DMA into scratch (fast, big descriptors),
                # then GpSimd scatters into the ones-interleaved layout.
                for qb in range(GROUP):
                    vscr = vscr_pool.tile([128, 1024], MMDT, tag="vscr")
                    for t4 in range(4):
                        off = qb * HG_LEN + HG_K + t4 * 128 * 256
                        nc.sync.dma_start(
                            vscr[:, t4 * 256 : (t4 + 1) * 256],
                            cc_outs[hg][off : off + 128 * 256].rearrange(
                                "(p f) -> p f", f=256
                            ),
                        )
                    for t4 in range(4):
                        kc = qb * 4 + t4
                        vdst = vones[
                            :,
                            kc * VSTR + 2 * hg * 192 + 64 : kc * VSTR
                            + 2 * hg * 192
                            + 128,
                        ]
                        nc.vector.tensor_copy(
                            bass.AP(
                                tensor=vdst.tensor,
                                offset=vdst.offset,
                                ap=[vdst.ap[0], [192, 2], [1, 128]],
                            ),
                            vscr[:, t4 * 256 : (t4 + 1) * 256].rearrange(
                                "p (c d) -> p c d", d=128
                            ),
                        )

            wout_sb = wo_pool.tile([128, DC * DIM], MMDT)
            for c in range(DC):
                nc.sync.dma_start(
                    wout_sb[:, c * DIM : (c + 1) * DIM],
                    w_out[c * 128 : (c + 1) * 128, :],
                )

            # ---- attention: per head, 16 key chunks in pairs -------------
            for h in range(HEADS):
                hp = (h % 2) * 64  # partition base of this head's rows
                hc = h // 2        # dim-chunk index of this head's rows
                po = pp.tile([128, 512], F32, tag="acc")
                for pr in range(KPAIR):
                    pss = ps_s.tile([128, 1024], F32)
                    for j in range(2):
                        kc = 2 * pr + j
                        qb, t4 = kc // 4, kc % 4
                        lhs_k = kt_sb[
                            hp : hp + 64,
                            (qb * DC + hc) * NQ + t4 * 128 : (qb * DC + hc) * NQ
                            + (t4 + 1) * 128,
                        ]
                        rhs_q = q_t[hp : hp + 64, hc * NQ : (hc + 1) * NQ]
                        nc.tensor.matmul(
                            pss[:, j * 512 : (j + 1) * 512],
                            lhs_k,
                            rhs_q,
                            start=True,
                            stop=True,
                        )
                    es = es_pool.tile([128, 1024], MMDT, tag="es")
                    nc.scalar.activation(es[:], pss[:], AF.Exp, scale=SCALE)
                    for j in range(2):
                        kc = 2 * pr + j
                        base = kc * VSTR + hc * 192 + (h % 2) * 128
                        nc.tensor.matmul(
                            po[:],
                            vones[:, base : base + 128],
                            es[:, j * 512 : (j + 1) * 512],
                            start=(pr == 0 and j == 0),
                            stop=(pr == KPAIR - 1 and j == 1),
                        )
                # data rows at hp, colsum rows at 64-hp
                # even heads: colsum rows 0:64, data 64:128; odd heads the
                # reverse. reciprocal_approx_fast only works at partition
                # base 0, so stage the colsum there and let the multiply
                # read it cross-base (legal: PSUM + SBUF inputs).
                cb, dp = hp, 64 - hp
                recip = rp_pool.tile([128, 1024], F32, tag="recip")
                nc.vector.tensor_copy(recip[0:64, 512:1024], po[cb : cb + 64, :])
                nc.vector.reciprocal_approx_fast(
                    recip[0:64, 0:512], recip[0:64, 512:1024]
                )
                nc.vector.tensor_mul(
                    att_t[hp : hp + 64, hc * NQ : (hc + 1) * NQ],
                    po[dp : dp + 64, :],
                    recip[0:64, 0:512],
                )

            if DEBUG_TAPS:
                nc.sync.dma_start(dbg_kt[:], kt_sb[:])
                nc.sync.dma_start(dbg_vo[:], vones[:])
                nc.sync.dma_start(dbg_qt[:], q_t[:])
                nc.sync.dma_start(dbg_at[:], att_t[:])

            # ---- output projection y^T = W_out^T @ att^T + b_out ---------
            for m in range(DC):
                py = pp.tile([128, 512], F32, tag="acc")
                for c in range(DC):
                    nc.tensor.matmul(
                        py[:],
                        wout_sb[:, c * DIM + m * 128 : c * DIM + (m + 1) * 128],
                        att_t[:, c * NQ : (c + 1) * NQ],
                        start=(c == 0),
                        stop=(c == DC - 1),
                    )
                y_sb = y_pool.tile([128, 512], F32, tag="y")
                nc.vector.tensor_scalar(
                    out=y_sb[:],
                    in0=py[:],
                    scalar1=bout_t[:, m : m + 1],
                    scalar2=None,
                    op0=ALU.add,
                )
                nc.sync.dma_start(out[m * 128 : (m + 1) * 128, :], y_sb[:])

    nc.compile()
    return nc


_NC_CACHE = None


def _get_nc():
    global _NC_CACHE
    if _NC_CACHE is None:
        _NC_CACHE = build_nc()
    return _NC_CACHE


def _make_in_maps(x, ln_gamma, ln_beta, W_qk, W_v, W_out, b_out):
    mmnp = mybir.dt.np(MMDT)
    wqk = np.ascontiguousarray(np.asarray(W_qk, dtype=np.float32)).astype(mmnp)
    wv = np.ascontiguousarray(np.asarray(W_v, dtype=np.float32)).astype(mmnp)
    wo = np.ascontiguousarray(np.asarray(W_out, dtype=np.float32)).astype(mmnp)
    gamma = np.ascontiguousarray(np.asarray(ln_gamma, dtype=np.float32))
    beta = np.ascontiguousarray(np.asarray(ln_beta, dtype=np.float32))
    bout = np.ascontiguousarray(np.asarray(b_out, dtype=np.float32))
    xf = np.asarray(x, dtype=np.float32)
    in_maps = []
    for i in range(NCORES):
        g, qq = i // GROUP, i % GROUP
        in_maps.append(
            {
                "x": np.ascontiguousarray(xf[g, qq * NQ : (qq + 1) * NQ, :]),
                "ln_gamma": gamma,
                "ln_beta": beta,
                "W_qk": wqk,
                "W_v": wv,
                "W_out": wo,
                "b_out": bout,
            }
        )
    return in_maps


def run(inputs: dict, trace: bool = False):
    """Run the distributed kernel; returns (full_output, BassKernelResults)."""
    nc = _get_nc()
    in_maps = _make_in_maps(**inputs)
    res = run_bass_kernel_spmd(
        nc, in_maps, core_ids=list(range(NCORES)), trace=trace
    )
    out_full = np.empty((B, N, DIM), dtype=np.float32)
    for i in range(NCORES):
        g, qq = i // GROUP, i % GROUP
        out_full[g, qq * NQ : (qq + 1) * NQ, :] = res.results[i]["out"].T
    return out_full, res


def kernel(**inputs) -> np.ndarray:
    out, _ = run(inputs, trace=False)
    return out


# revision 34
# speedup vs baseline: 1.1565x; 1.0685x over previous
"""Distributed Trainium2 Bass kernel for pre-LN multi-head attention.

Reference computation (per batch b of 2, seq n=2048, dim=1024, 16 heads x 64):
    xn = LayerNorm(x) * gamma + beta
    q, k = split(xn @ W_qk); v = xn @ W_v
    out = softmax(q k^T / 8) v  (per head)
    y = out @ W_out + b_out

Sharding: 8 cores = 2 batch groups x 4 sequence quarters. Core i owns batch
g=i//4, query tokens [qq*512, (qq+1)*512) with qq=i%4. Each core computes
LN + Q/K/V projections for its own 512 tokens, AllGathers K^T and V across
its 4-core group (full 2048-token K/V per batch), runs attention for its 512
queries over all 2048 keys (all 16 heads), and applies the output projection
with the full W_out — so the final output needs no inter-core reduction.
Host assembles the 8 per-core [1024, 512] y^T shards into [2, 2048, 1024].

Dataflow is kept "transposed" (feature dim on partitions, tokens on the free
axis) so every matmul contracts over the partition axis with 512-column
moving operands. Softmax normalization uses an ones-column block in the AV
stationary operand: each head's AV matmul emits colsum(exp S) on PSUM
partitions 0-63 and V^T exp(S^T) on partitions 64-127, so the divide is a
reciprocal + multiply on PSUM with no partition-axis reduction.

Matmul operands are stored bf16 (fp32 PSUM accumulation); LN statistics and
softmax run in fp32.
"""
import sys
import types

sys.path.insert(0, "/opt/trn_rl_repo")

# Register the NTFF profile hook that trn_boot skips when the image's antenv
# lacks axon_hooks, so run_bass_kernel_spmd(trace=True) can report exec time.
if "antenv.axon_hooks" not in sys.modules:
    try:
        from trn_agent_boot.trn_boot import _ntff_profile_via_ctypes

        _hook = _ntff_profile_via_ctypes("/opt/axon/libaxon_pjrt.so")
    except Exception:
        _hook = None
    _mod = types.ModuleType("antenv.axon_hooks")
    _mod.get_axon_ntff_profile_hook = lambda: _hook
    _mod.set_axon_ntff_profile_hook = lambda h: None
    sys.modules["antenv.axon_hooks"] = _mod

from contextlib import ExitStack

import ml_dtypes
import numpy as np
import concourse.bass as bass
import concourse.tile as tile
from concourse import bacc, mybir
from concourse.bass_utils import run_bass_kernel_spmd
from concourse.masks import make_identity

F32 = mybir.dt.float32
BF16 = mybir.dt.bfloat16
AF = mybir.ActivationFunctionType
ALU = mybir.AluOpType

B, N, DIM = 2, 2048, 1024
HEADS, DH = 16, 64
INNER = HEADS * DH  # 1024
SCALE = DH**-0.5
EPS = 1e-5

NCORES = 8
GROUP = 4          # cores per batch group (sequence quarters)
NQ = N // GROUP    # 512 local query tokens per core
DC = DIM // 128    # 8 dim chunks
KCH = N // 128     # 16 key chunks of 128 tokens
KPAIR = KCH // 2   # exp batches of 2 key chunks

MMDT = BF16        # matmul operand storage dtype
DEBUG_TAPS = False  # add debug DRAM outputs for kt/vones/qt/att

KT_BYTES = DIM * NQ          # elements in one K^T shard [1024, 512]
V_BYTES = NQ * INNER         # elements in one V shard  [512, 1024]
CC_LEN = KT_BYTES + V_BYTES  # per-rank collective payload
REPLICA_GROUPS = [[0, 1, 2, 3], [4, 5, 6, 7]]


def build_nc():
    nc = bacc.Bacc(num_devices=NCORES)

    x = nc.dram_tensor("x", [NQ, DIM], F32, kind="ExternalInput")
    ln_gamma = nc.dram_tensor("ln_gamma", [DIM], F32, kind="ExternalInput")
    ln_beta = nc.dram_tensor("ln_beta", [DIM], F32, kind="ExternalInput")
    w_qk = nc.dram_tensor("W_qk", [DIM, 2 * INNER], MMDT, kind="ExternalInput")
    w_v = nc.dram_tensor("W_v", [DIM, INNER], MMDT, kind="ExternalInput")
    w_out = nc.dram_tensor("W_out", [INNER, DIM], MMDT, kind="ExternalInput")
    b_out = nc.dram_tensor("b_out", [DIM], F32, kind="ExternalInput")
    out = nc.dram_tensor("out", [DIM, NQ], F32, kind="ExternalOutput")
    if DEBUG_TAPS:
        dbg_kt = nc.dram_tensor("dbg_kt", [128, 32 * NQ], MMDT, kind="ExternalOutput")
        dbg_vo = nc.dram_tensor(
            "dbg_vo", [128, KCH * 1536], MMDT, kind="ExternalOutput"
        )
        dbg_qt = nc.dram_tensor("dbg_qt", [128, DC * NQ], MMDT, kind="ExternalOutput")
        dbg_at = nc.dram_tensor("dbg_at", [128, DC * NQ], MMDT, kind="ExternalOutput")

    with tile.TileContext(nc) as tc, ExitStack() as ctx:
        pool = lambda name, bufs, **kw: ctx.enter_context(
            tc.tile_pool(name=name, bufs=bufs, **kw)
        )
        consts = pool("consts", 1)
        dram = pool("dram", 1, space="DRAM")
        qt_pool = pool("qt", 1)
        att_pool = pool("att", 1)
        small = pool("small", 8)
        stage = pool("stage", 3)
        pp = pool("pp", 2, space="PSUM")      # proj / outproj accumulators

        # ---- constants ---------------------------------------------------
        gamma_t = consts.tile([128, DC], F32)
        beta_t = consts.tile([128, DC], F32)
        bout_t = consts.tile([128, DC], F32)
        nc.sync.dma_start(gamma_t[:], ln_gamma.rearrange("(c p) -> p c", p=128))
        nc.sync.dma_start(beta_t[:], ln_beta.rearrange("(c p) -> p c", p=128))
        nc.sync.dma_start(bout_t[:], b_out.rearrange("(c p) -> p c", p=128))
        ident = consts.tile([128, 128], MMDT)
        make_identity(nc, ident[:])
        eps_sb = consts.tile([128, 1], F32)
        nc.vector.memset(eps_sb[:], EPS)

        # Collective payload split into 4 head-groups (4 heads each) so four
        # smaller AllGathers pipeline with projections and attention: group
        # hg carries K^T rows [hg*256,(hg+1)*256) and V cols likewise.
        HG_K = 2 * 128 * NQ      # K^T part elements per group
        HG_V = NQ * 256          # V part elements per group
        HG_LEN = HG_K + HG_V
        cc_ins = []
        cc_outs = []
        for hg in range(4):
            cc_i = dram.tile([HG_LEN], MMDT, name=f"cc_in{hg}")
            cc_o = dram.tile([GROUP * HG_LEN], MMDT, name=f"cc_out{hg}")
            cc_ins.append(cc_i)
            cc_outs.append(cc_o)

        # Q^T [1024, 512] as 8 chunks side by side: chunk m at cols m*512
        q_t = qt_pool.tile([128, DC * NQ], MMDT)
        # attention output^T [1024, 512], chunk c holds heads 2c, 2c+1
        att_t = att_pool.tile([128, DC * NQ], MMDT)

        # gathered K^T and interleaved V live in the outer scope so the
        # data-independent ones memsets run immediately on GpSimd without
        # waiting for the projection pools to release (they precede the
        # collectives on the GpSimd queue).
        kv = pool("kv", 1)
        kt_sb = kv.tile([128, GROUP * DC * NQ], MMDT)
        VSTR = 1600
        vones = kv.tile([128, KCH * VSTR], MMDT)
        for kc in range(KCH):
            ones_base = vones[:, kc * VSTR : kc * VSTR + 64]
            nc.gpsimd.memset(
                bass.AP(
                    tensor=ones_base.tensor,
                    offset=ones_base.offset,
                    ap=[ones_base.ap[0], [192, DC + 1], [1, 64]],
                ),
                1.0,
            )

        with ExitStack() as proj_ctx:
            ppool = lambda name, bufs, **kw: proj_ctx.enter_context(
                tc.tile_pool(name=name, bufs=bufs, **kw)
            )
            ptr = ppool("ptr", 2, space="PSUM")  # transpose targets
            xw = ppool("xw", 1)
            x_sb = xw.tile([128, GROUP * DIM], F32)
            xn_nat = xw.tile([128, GROUP * DIM], MMDT)
            xnt = xw.tile([128, DC * NQ], MMDT)
            wqk_sb = xw.tile([128, DC * 2 * INNER], MMDT)
            wv_sb = xw.tile([128, DC * INNER], MMDT)

            # x first so LayerNorm starts immediately; K-half of W_qk and
            # W_v next (feed the first head-group's projections); Q-half last.
            for t in range(GROUP):
                nc.sync.dma_start(
                    x_sb[:, t * DIM : (t + 1) * DIM],
                    x[t * 128 : (t + 1) * 128, :],
                )
            for c in range(DC):
                nc.sync.dma_start(
                    wqk_sb[:, c * 2 * INNER + INNER : (c + 1) * 2 * INNER],
                    w_qk[c * 128 : (c + 1) * 128, INNER:],
                )
                nc.sync.dma_start(
                    wv_sb[:, c * INNER : (c + 1) * INNER],
                    w_v[c * 128 : (c + 1) * 128, :],
                )
            for c in range(DC):
                nc.sync.dma_start(
                    wqk_sb[:, c * 2 * INNER : c * 2 * INNER + INNER],
                    w_qk[c * 128 : (c + 1) * 128, :INNER],
                )

            # ---- LayerNorm on the 4 local token chunks ------------------
            for t in range(GROUP):
                xt = x_sb[:, t * DIM : (t + 1) * DIM]
                xg = xt.rearrange("p (n s) -> p n s", s=512)
                stats = small.tile([128, 2, 6], F32)
                for sgi in range(2):
                    nc.vector.bn_stats(stats[:, sgi, :], xg[:, sgi, :])
                mv = small.tile([128, 2], F32)
                nc.vector.bn_aggr(mv[:], stats[:])
                rstd = small.tile([128, 1], F32)
                nc.scalar.activation(rstd[:], mv[:, 1:2], AF.Sqrt, bias=eps_sb[:])
                nc.vector.reciprocal(rstd[:], rstd[:])
                nc.vector.tensor_scalar(
                    out=xn_nat[:, t * DIM : (t + 1) * DIM],
                    in0=xt,
                    scalar1=mv[:, 0:1],
                    scalar2=rstd[:],
                    op0=ALU.subtract,
                    op1=ALU.mult,
                )

            # ---- transpose xn to [dim, tokens], fusing gamma/beta -------
            # c-major so xnT chunk 0 completes first and projections start
            # before the whole transpose sweep finishes
            for c in range(DC):
                for t in range(GROUP):
                    pt = ptr.tile([128, 128], MMDT)
                    nc.tensor.transpose(
                        pt[:],
                        xn_nat[:, t * DIM + c * 128 : t * DIM + (c + 1) * 128],
                        ident[:],
                    )
                    nc.vector.tensor_scalar(
                        out=xnt[:, c * NQ + t * 128 : c * NQ + (t + 1) * 128],
                        in0=pt[:],
                        scalar1=gamma_t[:, c : c + 1],
                        scalar2=beta_t[:, c : c + 1],
                        op0=ALU.mult,
                        op1=ALU.add,
                    )

            # ---- per head-group: K^T + V projections, then its AllGather -
            for hg in range(4):
                for mg in range(2):
                    m = DC + 2 * hg + mg  # K^T row block (W_qk col block)
                    pq = pp.tile([128, 512], F32, tag="acc")
                    for c in range(DC):
                        nc.tensor.matmul(
                            pq[:],
                            wqk_sb[:, c * 2 * INNER + m * 128 : c * 2 * INNER + (m + 1) * 128],
                            xnt[:, c * NQ : (c + 1) * NQ],
                            start=(c == 0),
                            stop=(c == DC - 1),
                        )
                    kst = stage.tile([128, 512], MMDT, tag="stg")
                    nc.scalar.copy(kst[:], pq[:])
                    koff = mg * 128 * NQ
                    nc.sync.dma_start(
                        cc_ins[hg][koff : koff + 128 * NQ].rearrange(
                            "(p f) -> p f", f=NQ
                        ),
                        kst[:],
                    )
                for t in range(GROUP):
                    pv = pp.tile([128, 512], F32, tag="acc")
                    for c in range(DC):
                        nc.tensor.matmul(
                            pv[:, 0:256],
                            xnt[:, c * NQ + t * 128 : c * NQ + (t + 1) * 128],
                            wv_sb[:, c * INNER + hg * 256 : c * INNER + (hg + 1) * 256],
                            start=(c == 0),
                            stop=(c == DC - 1),
                        )
                    vst = stage.tile([128, 512], MMDT, tag="stg")
                    nc.scalar.copy(vst[:, 0:256], pv[:, 0:256])
                    voff = HG_K + t * 128 * 256
                    nc.sync.dma_start(
                        cc_ins[hg][voff : voff + 128 * 256].rearrange(
                            "(p f) -> p f", f=256
                        ),
                        vst[:, 0:256],
                    )
                nc.gpsimd.collective_compute(
                    "AllGather",
                    ALU.bypass,
                    replica_groups=REPLICA_GROUPS,
                    ins=[cc_ins[hg][:].opt()],
                    outs=[cc_outs[hg][:].opt()],
                )

            # ---- Q^T (cols 0..1023 of W_qk), overlaps the AllGathers -----
            for m in range(DC):
                pq = pp.tile([128, 512], F32, tag="acc")
                for c in range(DC):
                    nc.tensor.matmul(
                        pq[:],
                        wqk_sb[:, c * 2 * INNER + m * 128 : c * 2 * INNER + (m + 1) * 128],
                        xnt[:, c * NQ : (c + 1) * NQ],
                        start=(c == 0),
                        stop=(c == DC - 1),
                    )
                nc.vector.tensor_copy(q_t[:, m * NQ : (m + 1) * NQ], pq[:])

        # ---- attention-phase SBUF (proj pools released) ------------------
        with ExitStack() as att_ctx:
            apool = lambda name, bufs, **kw: att_ctx.enter_context(
                tc.tile_pool(name=name, bufs=bufs, **kw)
            )
            wo_pool = apool("wo", 1)
            es_pool = apool("es", 4)
            vscr_pool = apool("vscr", 3)
            rp_pool = apool("rp", 2)
            y_pool = apool("y", 2)
            ps_s = apool("ps_s", 3, space="PSUM")

            # (kt_sb / vones / ones memsets moved to the outer scope)
            # per head-group loads, in attention consumption order
            for hg in range(4):
                for qb in range(GROUP):
                    for mg in range(2):
                        src = cc_outs[hg][
                            qb * HG_LEN + mg * 128 * NQ : qb * HG_LEN
                            + (mg + 1) * 128 * NQ
                        ].rearrange("(p f) -> p f", f=NQ)
                        nc.sync.dma_start(
                            kt_sb[
                                :,
                                (qb * DC + 2 * hg + mg) * NQ : (qb * DC + 2 * hg + mg + 1)
                                * NQ,
                            ],
                            src,
                        )
                # V: contiguous DMA into scratch (fast, big descriptors),
                # then GpSimd scatters into the ones-interleaved layout.
                for qb in range(GROUP):
                    vscr = vscr_pool.tile([128, 1024], MMDT, tag="vscr")
                    for t4 in range(4):
                        off = qb * HG_LEN + HG_K + t4 * 128 * 256
                        nc.sync.dma_start(
                            vscr[:, t4 * 256 : (t4 + 1) * 256],
                            cc_outs[hg][off : off + 128 * 256].rearrange(
                                "(p f) -> p f", f=256
                            ),
                        )
                    for t4 in range(4):
                        kc = qb * 4 + t4
                        vdst = vones[
                            :,
                            kc * VSTR + 2 * hg * 192 + 64 : kc * VSTR
                            + 2 * hg * 192
                            + 128,
                        ]
                        nc.vector.tensor_copy(
                            bass.AP(
                                tensor=vdst.tensor,
                                offset=vdst.offset,
                                ap=[vdst.ap[0], [192, 2], [1, 128]],
                            ),
                            vscr[:, t4 * 256 : (t4 + 1) * 256].rearrange(
                                "p (c d) -> p c d", d=128
                            ),
                        )

            wout_sb = wo_pool.tile([128, DC * DIM], MMDT)
            for c in range(DC):
                nc.sync.dma_start(
                    wout_sb[:, c * DIM : (c + 1) * DIM],
                    w_out[c * 128 : (c + 1) * 128, :],
                )

            # ---- attention: per head, 16 key chunks in pairs -------------
            for h in range(HEADS):
                hp = (h % 2) * 64  # partition base of this head's rows
                hc = h // 2        # dim-chunk index of this head's rows
                po = pp.tile([128, 512], F32, tag="acc")
                for pr in range(KPAIR):
                    pss = ps_s.tile([128, 1024], F32)
                    for j in range(2):
                        kc = 2 * pr + j
                        qb, t4 = kc // 4, kc % 4
                        lhs_k = kt_sb[
                            hp : hp + 64,
                            (qb * DC + hc) * NQ + t4 * 128 : (qb * DC + hc) * NQ
                            + (t4 + 1) * 128,
                        ]
                        rhs_q = q_t[hp : hp + 64, hc * NQ : (hc + 1) * NQ]
                        nc.tensor.matmul(
                            pss[:, j * 512 : (j + 1) * 512],
                            lhs_k,
                            rhs_q,
                            start=True,
                            stop=True,
                        )
                    es = es_pool.tile([128, 1024], MMDT, tag="es")
                    nc.scalar.activation(es[:], pss[:], AF.Exp, scale=SCALE)
                    for j in range(2):
                        kc = 2 * pr + j
                        base = kc * VSTR + hc * 192 + (h % 2) * 128
                        nc.tensor.matmul(
                            po[:],
                            vones[:, base : base + 128],
                            es[:, j * 512 : (j + 1) * 512],
                            start=(pr == 0 and j == 0),
                            stop=(pr == KPAIR - 1 and j == 1),
                        )
                # data rows at hp, colsum rows at 64-hp
                # even heads: colsum rows 0:64, data 64:128; odd heads the
                # reverse. reciprocal_approx_fast only works at partition
                # base 0, so stage the colsum there and let the multiply
                # read it cross-base (legal: PSUM + SBUF inputs).
                cb, dp = hp, 64 - hp
                recip = rp_pool.tile([128, 1024], F32, tag="recip")
                nc.vector.tensor_copy(recip[0:64, 512:1024], po[cb : cb + 64, :])
                nc.vector.reciprocal_approx_fast(
                    recip[0:64, 0:512], recip[0:64, 512:1024]
                )
                nc.vector.tensor_mul(
                    att_t[hp : hp + 64, hc * NQ : (hc + 1) * NQ],
                    po[dp : dp + 64, :],
                    recip[0:64, 0:512],
                )

            if DEBUG_TAPS:
                nc.sync.dma_start(dbg_kt[:], kt_sb[:])
                nc.sync.dma_start(dbg_vo[:], vones[:])
                nc.sync.dma_start(dbg_qt[:], q_t[:])
                nc.sync.dma_start(dbg_at[:], att_t[:])

            # ---- output projection y^T = W_out^T @ att^T + b_out ---------
            for m in range(DC):
                py = pp.tile([128, 512], F32, tag="acc")
                for c in range(DC):
                    nc.tensor.matmul(
                        py[:],
                        wout_sb[:, c * DIM + m * 128 : c * DIM + (m + 1) * 128],
                        att_t[:, c * NQ : (c + 1) * NQ],
                        start=(c == 0),
                        stop=(c == DC - 1),
                    )
                y_sb = y_pool.tile([128, 512], F32, tag="y")
                nc.vector.tensor_scalar(
                    out=y_sb[:],
                    in0=py[:],
                    scalar1=bout_t[:, m : m + 1],
                    scalar2=None,
                    op0=ALU.add,
                )
                nc.sync.dma_start(out[m * 128 : (m + 1) * 128, :], y_sb[:])

    nc.compile()
    return nc


_NC_CACHE = None


def _get_nc():
    global _NC_CACHE
    if _NC_CACHE is None:
        _NC_CACHE = build_nc()
    return _NC_CACHE


def _make_in_maps(x, ln_gamma, ln_beta, W_qk, W_v, W_out, b_out):
    mmnp = mybir.dt.np(MMDT)
    wqk = np.ascontiguousarray(np.asarray(W_qk, dtype=np.float32)).astype(mmnp)
    wv = np.ascontiguousarray(np.asarray(W_v, dtype=np.float32)).astype(mmnp)
    wo = np.ascontiguousarray(np.asarray(W_out, dtype=np.float32)).astype(mmnp)
    gamma = np.ascontiguousarray(np.asarray(ln_gamma, dtype=np.float32))
    beta = np.ascontiguousarray(np.asarray(ln_beta, dtype=np.float32))
    bout = np.ascontiguousarray(np.asarray(b_out, dtype=np.float32))
    xf = np.asarray(x, dtype=np.float32)
    in_maps = []
    for i in range(NCORES):
        g, qq = i // GROUP, i % GROUP
        in_maps.append(
            {
                "x": np.ascontiguousarray(xf[g, qq * NQ : (qq + 1) * NQ, :]),
                "ln_gamma": gamma,
                "ln_beta": beta,
                "W_qk": wqk,
                "W_v": wv,
                "W_out": wo,
                "b_out": bout,
            }
        )
    return in_maps


def run(inputs: dict, trace: bool = False):
    """Run the distributed kernel; returns (full_output, BassKernelResults)."""
    nc = _get_nc()
    in_maps = _make_in_maps(**inputs)
    res = run_bass_kernel_spmd(
        nc, in_maps, core_ids=list(range(NCORES)), trace=trace
    )
    out_full = np.empty((B, N, DIM), dtype=np.float32)
    for i in range(NCORES):
        g, qq = i // GROUP, i % GROUP
        out_full[g, qq * NQ : (qq + 1) * NQ, :] = res.results[i]["out"].T
    return out_full, res


def kernel(**inputs) -> np.ndarray:
    out, _ = run(inputs, trace=False)
    return out
